# revision 19
# baseline (speedup 1.0000x reference)
"""Causal self-attention (B=2, S=2048, D=2048, H=16) on 8 TRN2 NeuronCores.

Sharding: 2 batches x 4 head-groups.  Core c handles batch c//4 and heads
[4*(c%4) .. 4*(c%4)+3].  Per core:
  phase 1: qT/kT (transposed) + v projections from host-pre-transposed xT
  phase 2: causal attention per (q-block, head), q-blocks processed in
           descending order so each q-block's AllGather (groups of 4) of the
           normalized yT overlaps the remaining attention + out-projection
  phase 3: out projection of the core's 512 output columns + bias
Softmax uses exp without max subtraction (logits are O(8) here); denominators
are accumulated on DVE (elementwise over k-tiles) then reduced across
partitions with a single ones-matmul, inverted with reciprocal_approx_fast.

Compute is bf16 with fp32 PSUM accumulation; measured l2 rel err vs the fp32
reference is ~5.5e-3.
"""

import numpy as np
import ml_dtypes

B, S, D = 2, 2048, 2048
H, HD = 16, 128
HLOC = 4           # heads per core
CW = HLOC * HD     # 512: per-core q/k/v width and out-column width
QB = 4             # q blocks of 512
DT = 16            # d tiles of 128
TB = 4             # token blocks of 512
SCALE = 1.0 / float(np.sqrt(HD))
GROUPS = [[0, 1, 2, 3], [4, 5, 6, 7]]

_cache = {}


def _build():
    import concourse.tile as tile
    import concourse.mybir as mybir
    from concourse import bacc

    BF = mybir.dt.bfloat16
    F32 = mybir.dt.float32

    nc = bacc.Bacc("TRN2", target_bir_lowering=False, debug=False, num_devices=8)

    # Inputs (per-core shards, host-prepared)
    xT = nc.dram_tensor("xT", [D, S], BF, kind="ExternalInput")            # x[batch].T
    wqk = nc.dram_tensor("wqk", [DT, 8, 128, 128], BF, kind="ExternalInput")  # (dt, ct, d, col)
    wv = nc.dram_tensor("wv", [DT, 128, CW], BF, kind="ExternalInput")     # (dt, d, vcol)
    bqk = nc.dram_tensor("bqk", [8, 128, 1], F32, kind="ExternalInput")
    bv = nc.dram_tensor("bv", [1, CW], F32, kind="ExternalInput")
    wout = nc.dram_tensor("wout", [D, CW], BF, kind="ExternalInput")
    bout = nc.dram_tensor("bout", [1, CW], F32, kind="ExternalInput")
    out = nc.dram_tensor("out", [S, CW], F32, kind="ExternalOutput")

    # two AG halves: hi = q-blocks {2,3} (tokens 1024:2048), lo = {0,1}
    ag_in = [nc.dram_tensor(f"ag_in{i}", [CW, 1024], BF, kind="Internal")
             for i in range(2)]
    ag_out = [nc.dram_tensor(f"ag_out{i}", [D, 1024], BF, kind="Internal")
              for i in range(2)]

    with tile.TileContext(nc) as tc:
        with (
            tc.tile_pool(name="const", bufs=1) as constp,
            tc.tile_pool(name="pers", bufs=1) as pers,
            tc.tile_pool(name="work", bufs=2) as work,
            tc.tile_pool(name="psum", bufs=2, space="PSUM") as psum,
        ):
            # ---- constants ----
            ones = constp.tile([128, 1], BF, name="ones")
            nc.gpsimd.memset(ones[:], 1.0)

            # Pair masks for the 4 diagonal k-subtiles, packed two subtiles
            # wide: pairmask[m][:, 512*sub + qq] keeps where
            # qq >= kk + 128*(2m+sub).
            pairmasks = []
            for pm in range(2):
                m = constp.tile([128, 1024], BF, name=f"pmask{pm}",
                                tag=f"pmask{pm}")
                nc.gpsimd.memset(m[:], 1.0)
                for sub in range(2):
                    nc.gpsimd.affine_select(
                        out=m[:, sub * 512:(sub + 1) * 512],
                        in_=m[:, sub * 512:(sub + 1) * 512],
                        compare_op=mybir.AluOpType.is_ge, fill=0.0,
                        base=-128 * (2 * pm + sub), channel_multiplier=-1,
                        pattern=[[1, 512]],
                    )
                pairmasks.append(m)

            bout_sb = constp.tile([1, CW], F32, name="bout_sb")
            nc.sync.dma_start(bout_sb[:], bout[:])
            bias_bc = constp.tile([128, CW], F32, name="bias_bc")
            nc.gpsimd.partition_broadcast(bias_bc[:], bout_sb[:], channels=128)

            bv_sb = constp.tile([1, CW], F32, name="bv_sb")
            nc.sync.dma_start(bv_sb[:], bv[:])
            vbias_bc = constp.tile([128, CW], F32, name="vbias_bc")
            nc.gpsimd.partition_broadcast(vbias_bc[:], bv_sb[:], channels=128)

            bqk_sb = []
            for ct in range(8):
                t = constp.tile([128, 1], F32, name=f"bqk{ct}", tag=f"bqk{ct}")
                nc.sync.dma_start(t[:], bqk[ct])
                bqk_sb.append(t)

            # ---- persistent SBUF tensors ----
            qT = [pers.tile([128, S], BF, name=f"qT{h}", tag=f"qT{h}")
                  for h in range(HLOC)]
            kT = [pers.tile([128, S], BF, name=f"kT{h}", tag=f"kT{h}")
                  for h in range(HLOC)]
            vt = [pers.tile([128, CW], BF, name=f"v{t}", tag=f"v{t}")
                  for t in range(16)]
            yT = [pers.tile([128, S], BF, name=f"yT{h}", tag=f"yT{h}")
                  for h in range(HLOC)]

            # ---- input loads: wv panels first (v-projection can then start
            # as soon as the first xT token block lands), then xT tb-major ----
            wv_sb = []
            for dt in range(DT):
                wvp = work.tile([128, CW], BF, name=f"wvp{dt}", tag="p512",
                                bufs=17)
                nc.sync.dma_start(wvp[:], wv[dt])
                wv_sb.append(wvp)
            xt_tiles = {}
            for tb in range(TB):
                for dt in range(DT):
                    t = work.tile([128, 512], BF, name=f"xt_{dt}_{tb}",
                                  tag="xT", bufs=68)
                    nc.sync.dma_start(
                        t[:], xT[dt * 128:(dt + 1) * 128, tb * 512:(tb + 1) * 512])
                    xt_tiles[(dt, tb)] = t

            # ---- v projection: v[t] = x @ wv  ([tok, vcol], xT stationary) ----
            for t in range(16):
                tb, j = t // 4, t % 4
                acc = psum.tile([128, CW], F32, name="acc_v", tag="acc", bufs=2)
                for dt in range(DT):
                    nc.tensor.matmul(
                        acc[:],
                        xt_tiles[(dt, tb)][:, j * 128:(j + 1) * 128],
                        wv_sb[dt][:],
                        start=(dt == 0), stop=(dt == DT - 1),
                    )
                nc.vector.tensor_tensor(vt[t][:], acc[:], vbias_bc[:],
                                        mybir.AluOpType.add)

            # ---- q/k projections: qT/kT[ct][col, tok] (w stationary) ----
            for ct in range(8):
                wts = []
                for dt in range(DT):
                    wt = work.tile([128, 128], BF, name=f"w_{ct}_{dt}",
                                   tag="w", bufs=24)
                    nc.sync.dma_start(wt[:], wqk[dt, ct])
                    wts.append(wt)
                dest = qT[ct] if ct < 4 else kT[ct - 4]
                for tb in range(TB):
                    acc = psum.tile([128, 512], F32, name="acc_qk", tag="acc",
                                    bufs=2)
                    for dt in range(DT):
                        nc.tensor.matmul(
                            acc[:], wts[dt][:], xt_tiles[(dt, tb)][:],
                            start=(dt == 0), stop=(dt == DT - 1),
                        )
                    nc.scalar.activation(
                        dest[:, tb * 512:(tb + 1) * 512], acc[:],
                        mybir.ActivationFunctionType.Identity,
                        bias=bqk_sb[ct][:], scale=1.0,
                    )

            # ---- attention + chunked AllGather (qb descending), with the
            # out-projection of chunk qb+1 interleaved one AG behind so the
            # sync DMA queue never head-of-line blocks on a collective ----
            wout_sb = []
            for dt in range(DT):
                t = work.tile([128, CW], BF, name=f"wout{dt}", tag="p512",
                              bufs=17)
                nc.sync.dma_start(t[:], wout[dt * 128:(dt + 1) * 128, :])
                wout_sb.append(t)

            def attention_chunk(qb):
                nk = 4 * qb + 4
                for h in range(HLOC):
                    y_ps = psum.tile([128, 512], F32, name="y_ps", tag="y")
                    esum = work.tile([128, 1024], BF, name="esum", tag="esum",
                                     bufs=2)
                    # k-subtiles in pairs; PV/sum of pair pr-1 issue after
                    # QK/exp of pair pr so the PE never waits on a fresh exp
                    prev = None

                    def flush(prev_pair):
                        e, pr = prev_pair
                        for s_ in range(2):
                            kt = 2 * pr + s_
                            nc.tensor.matmul(
                                y_ps[:],
                                vt[kt][:, h * 128:(h + 1) * 128],
                                e[:, s_ * 512:(s_ + 1) * 512],
                                start=(kt == 0), stop=(kt == nk - 1),
                            )
                        if pr == 0:
                            nc.vector.tensor_copy(esum[:], e[:])
                        else:
                            nc.vector.tensor_tensor(esum[:], esum[:], e[:],
                                                    mybir.AluOpType.add)

                    for pr in range(nk // 2):
                        sc = psum.tile([128, 1024], F32, name="sc", tag="s",
                                       bufs=2)
                        for s_ in range(2):
                            kt = 2 * pr + s_
                            nc.tensor.matmul(
                                sc[:, s_ * 512:(s_ + 1) * 512],
                                kT[h][:, kt * 128:(kt + 1) * 128],
                                qT[h][:, qb * 512:(qb + 1) * 512],
                                start=True, stop=True,
                            )
                        e = work.tile([128, 1024], BF, name="expT", tag="expT",
                                      bufs=4)
                        nc.scalar.activation(
                            e[:], sc[:], mybir.ActivationFunctionType.Exp,
                            scale=SCALE,
                        )
                        pm = pr - (nk // 2 - 2)
                        if pm >= 0:
                            nc.vector.tensor_tensor(e[:], e[:],
                                                    pairmasks[pm][:],
                                                    mybir.AluOpType.mult)
                        if prev is not None:
                            flush(prev)
                        prev = (e, pr)
                    flush(prev)

                    esum_f = work.tile([128, 512], BF, name="esum_f",
                                       tag="esum_f", bufs=2)
                    nc.vector.tensor_tensor(esum_f[:], esum[:, 0:512],
                                            esum[:, 512:1024],
                                            mybir.AluOpType.add)
                    sum_ps = psum.tile([1, 512], F32, name="sum_ps", tag="y")
                    nc.tensor.matmul(sum_ps[:], ones[:], esum_f[:],
                                     start=True, stop=True)
                    recip = work.tile([1, 512], F32, name="recip", tag="recip",
                                      bufs=2)
                    nc.vector.reciprocal_approx_fast(recip[:], sum_ps[:])
                    rbc = work.tile([128, 512], F32, name="rbc", tag="rbc",
                                    bufs=2)
                    nc.gpsimd.partition_broadcast(rbc[:], recip[:], channels=128)
                    nc.vector.tensor_tensor(
                        yT[h][:, qb * 512:(qb + 1) * 512], y_ps[:], rbc[:],
                        mybir.AluOpType.mult,
                    )
                    half, co = qb // 2, (qb % 2) * 512
                    nc.sync.dma_start(
                        ag_in[half][h * 128:(h + 1) * 128, co:co + 512],
                        yT[h][:, qb * 512:(qb + 1) * 512],
                    )

            def trigger_ag(half):
                nc.gpsimd.collective_compute(
                    "AllGather", mybir.AluOpType.bypass,
                    replica_groups=GROUPS,
                    ins=[ag_in[half].ap()], outs=[ag_out[half].ap()],
                )

            def outproj_half(half):
                ygt = {}
                for dt in range(DT):
                    for hc in range(2):
                        t = work.tile([128, 512], BF,
                                      name=f"ygT_{half}_{dt}_{hc}",
                                      tag="xT", bufs=68)
                        nc.sync.dma_start(
                            t[:], ag_out[half][dt * 128:(dt + 1) * 128,
                                               hc * 512:(hc + 1) * 512])
                        ygt[(dt, hc)] = t
                for j in range(8):
                    acc = psum.tile([128, CW], F32, name="acc_o", tag="acc",
                                    bufs=2)
                    for dt in range(DT):
                        nc.tensor.matmul(
                            acc[:],
                            ygt[(dt, j // 4)][:, (j % 4) * 128:(j % 4 + 1) * 128],
                            wout_sb[dt][:],
                            start=(dt == 0), stop=(dt == DT - 1),
                        )
                    osb = work.tile([128, CW], F32, name="osb", tag="osb",
                                    bufs=3)
                    nc.vector.tensor_tensor(osb[:], acc[:], bias_bc[:],
                                            mybir.AluOpType.add)
                    tt = half * 8 + j
                    nc.sync.dma_start(out[tt * 128:(tt + 1) * 128, :], osb[:])

            attention_chunk(3)
            attention_chunk(2)
            trigger_ag(1)
            attention_chunk(1)
            attention_chunk(0)
            trigger_ag(0)
            outproj_half(1)
            outproj_half(0)

    nc.compile()
    return nc


def _prep_inputs(x, w_qkv, b_qkv, w_out, b_out):
    """Host-side sharding/layout. Returns in_maps for the 8 cores."""
    bf16 = ml_dtypes.bfloat16
    x = np.asarray(x, dtype=np.float32)
    w_qkv = np.asarray(w_qkv, dtype=np.float32)
    b_qkv = np.asarray(b_qkv, dtype=np.float32)
    w_out = np.asarray(w_out, dtype=np.float32)
    b_out = np.asarray(b_out, dtype=np.float32)

    xT_b = [np.ascontiguousarray(x[b].T).astype(bf16) for b in range(B)]

    in_maps = []
    for c in range(8):
        b, g = c // 4, c % 4
        cols = slice(CW * g, CW * (g + 1))
        wq = w_qkv[:, 0 * D:1 * D][:, cols]
        wk = w_qkv[:, 1 * D:2 * D][:, cols]
        wv_ = w_qkv[:, 2 * D:3 * D][:, cols]
        # (dt, ct, d, col) for q(0-3) then k(4-7), 128-col tiles
        wqk = np.concatenate([wq, wk], axis=1)            # [D, 1024]
        wqk = wqk.reshape(DT, 128, 8, 128).transpose(0, 2, 1, 3)
        wqk = np.ascontiguousarray(wqk).astype(bf16)
        wv_t = np.ascontiguousarray(wv_.reshape(DT, 128, CW)).astype(bf16)

        bq = b_qkv[0 * D:1 * D][cols]
        bk = b_qkv[1 * D:2 * D][cols]
        bv_ = b_qkv[2 * D:3 * D][cols]
        bqk = np.concatenate([bq, bk]).reshape(8, 128, 1).astype(np.float32)

        in_maps.append({
            "xT": xT_b[b],
            "wqk": wqk,
            "wv": wv_t,
            "bqk": np.ascontiguousarray(bqk),
            "bv": np.ascontiguousarray(bv_.reshape(1, CW)),
            "wout": np.ascontiguousarray(w_out[:, cols]).astype(bf16),
            "bout": np.ascontiguousarray(b_out[cols].reshape(1, CW)),
        })
    return in_maps


def kernel(x, w_qkv, b_qkv, w_out, b_out, _trace=False, _trace_kwargs=None):
    from concourse.bass_utils import run_bass_kernel_spmd

    if "nc" not in _cache:
        _cache["nc"] = _build()
    nc = _cache["nc"]

    in_maps = _prep_inputs(x, w_qkv, b_qkv, w_out, b_out)
    res = run_bass_kernel_spmd(
        nc, in_maps, core_ids=list(range(8)),
        trace=_trace, **(_trace_kwargs or {}),
    )

    out = np.empty((B, S, D), dtype=np.float32)
    for c in range(8):
        b, g = c // 4, c % 4
        out[b][:, CW * g:CW * (g + 1)] = res.results[c]["out"]
    kernel.last_result = res
    return out


# revision 20
# speedup vs baseline: 1.1169x; 1.1169x over previous
"""Causal self-attention (B=2, S=2048, D=2048, H=16) on 8 TRN2 NeuronCores.

Sharding: 2 batches x 4 head-groups.  Core c handles batch c//4 and heads
[4*(c%4) .. 4*(c%4)+3]; each core produces output columns [512*(c%4) ...].

Per core, head-pipelined so the AllGathers hide under compute:
  v projection (all 4 heads), then for each local head h:
    q/k projection (transposed layout) -> causal attention over 4 q-blocks
    -> AllGather (groups of 4) of that head's normalized yT (bf16)
    -> out-projection partial pass for head-chunk h-1 (one AG behind)
The out projection accumulates head-chunk partials (bf16 SBUF) with w_out
rows host-permuted to match the AG's rank-major row order; only the last
head's AG + partial pass is exposed comm.

Softmax uses exp without max subtraction (logits are O(8) here); denominators
are accumulated on DVE over k-tile pairs then reduced across partitions with
a single ones-matmul, inverted with reciprocal_approx_fast.

Compute is bf16 with fp32 PSUM accumulation; measured l2 rel err vs the fp32
reference is ~5.5e-3.
"""

import numpy as np
import ml_dtypes

B, S, D = 2, 2048, 2048
H, HD = 16, 128
HLOC = 4           # heads per core
CW = HLOC * HD     # 512: per-core v width and out-column width
QB = 4             # q blocks of 512
DT = 16            # d tiles of 128
TB = 4             # token blocks of 512
SCALE = 1.0 / float(np.sqrt(HD))
GROUPS = [[0, 1, 2, 3], [4, 5, 6, 7]]

_cache = {}


def _build():
    import concourse.tile as tile
    import concourse.mybir as mybir
    from concourse import bacc

    BF = mybir.dt.bfloat16
    F32 = mybir.dt.float32

    nc = bacc.Bacc("TRN2", target_bir_lowering=False, debug=False, num_devices=8)

    # Inputs (per-core shards, host-prepared)
    xT = nc.dram_tensor("xT", [D, S], BF, kind="ExternalInput")          # x[batch].T
    wqk = nc.dram_tensor("wqk", [HLOC, 2, DT, 128, 128], BF, kind="ExternalInput")
    wv = nc.dram_tensor("wv", [DT, 128, CW], BF, kind="ExternalInput")
    bqk = nc.dram_tensor("bqk", [HLOC, 2, 128, 1], F32, kind="ExternalInput")
    bv = nc.dram_tensor("bv", [1, CW], F32, kind="ExternalInput")
    # w_out rows permuted: wout[h][i] = w_out[512*i + 128*h : +128, cols]
    wout = nc.dram_tensor("wout", [HLOC, 4, 128, CW], BF, kind="ExternalInput")
    bout = nc.dram_tensor("bout", [1, CW], F32, kind="ExternalInput")
    out = nc.dram_tensor("out", [S, CW], F32, kind="ExternalOutput")

    ag_in = [nc.dram_tensor(f"ag_in{h}", [128, S], BF, kind="Internal")
             for h in range(HLOC)]
    ag_out = [nc.dram_tensor(f"ag_out{h}", [512, S], BF, kind="Internal")
              for h in range(HLOC)]

    with tile.TileContext(nc) as tc:
        with (
            tc.tile_pool(name="const", bufs=1) as constp,
            tc.tile_pool(name="pers", bufs=1) as pers,
            tc.tile_pool(name="work", bufs=2) as work,
            tc.tile_pool(name="psum", bufs=2, space="PSUM") as psum,
        ):
            # ---- constants ----
            ones = constp.tile([128, 1], BF, name="ones")
            nc.gpsimd.memset(ones[:], 1.0)

            # Pair masks for the 4 diagonal k-subtiles, packed two subtiles
            # wide: pairmask[m][:, 512*sub + qq] keeps where
            # qq >= kk + 128*(2m+sub).
            pairmasks = []
            for pm in range(2):
                m = constp.tile([128, 1024], BF, name=f"pmask{pm}",
                                tag=f"pmask{pm}")
                nc.gpsimd.memset(m[:], 1.0)
                for sub in range(2):
                    nc.gpsimd.affine_select(
                        out=m[:, sub * 512:(sub + 1) * 512],
                        in_=m[:, sub * 512:(sub + 1) * 512],
                        compare_op=mybir.AluOpType.is_ge, fill=0.0,
                        base=-128 * (2 * pm + sub), channel_multiplier=-1,
                        pattern=[[1, 512]],
                    )
                pairmasks.append(m)

            bout_sb = constp.tile([1, CW], F32, name="bout_sb")
            nc.sync.dma_start(bout_sb[:], bout[:])
            bias_bc = constp.tile([128, CW], F32, name="bias_bc")
            nc.gpsimd.partition_broadcast(bias_bc[:], bout_sb[:], channels=128)

            bv_sb = constp.tile([1, CW], F32, name="bv_sb")
            nc.sync.dma_start(bv_sb[:], bv[:])
            vbias_bc = constp.tile([128, CW], F32, name="vbias_bc")
            nc.gpsimd.partition_broadcast(vbias_bc[:], bv_sb[:], channels=128)

            bqk_sb = {}
            for h in range(HLOC):
                for qk in range(2):
                    t = constp.tile([128, 1], F32, name=f"bqk{h}{qk}",
                                    tag=f"bqk{h}{qk}")
                    nc.sync.dma_start(t[:], bqk[h, qk])
                    bqk_sb[(h, qk)] = t

            # ---- persistent v tiles ----
            vt = [pers.tile([128, CW], BF, name=f"v{t}", tag=f"v{t}")
                  for t in range(16)]

            # ---- loads: wv panels first, then xT tb-major ----
            wv_sb = []
            for dt in range(DT):
                wvp = work.tile([128, CW], BF, name=f"wvp{dt}", tag="p512",
                                bufs=17)
                nc.sync.dma_start(wvp[:], wv[dt])
                wv_sb.append(wvp)
            xt_tiles = {}
            for tb in range(TB):
                for dt in range(DT):
                    t = work.tile([128, 512], BF, name=f"xt_{dt}_{tb}",
                                  tag="xT", bufs=68)
                    nc.sync.dma_start(
                        t[:], xT[dt * 128:(dt + 1) * 128, tb * 512:(tb + 1) * 512])
                    xt_tiles[(dt, tb)] = t

            # ---- v projection: v[t] = x @ wv  ([tok, vcol], xT stationary) ----
            for t in range(16):
                tb, j = t // 4, t % 4
                acc = psum.tile([128, CW], F32, name="acc_v", tag="acc", bufs=2)
                for dt in range(DT):
                    nc.tensor.matmul(
                        acc[:],
                        xt_tiles[(dt, tb)][:, j * 128:(j + 1) * 128],
                        wv_sb[dt][:],
                        start=(dt == 0), stop=(dt == DT - 1),
                    )
                nc.vector.tensor_tensor(vt[t][:], acc[:], vbias_bc[:],
                                        mybir.AluOpType.add)

            # ---- per-head q/k projection ([col, tok] transposed) ----
            def qk_proj(h):
                dests = {}
                for qk in range(2):
                    wts = []
                    for dt in range(DT):
                        wt = work.tile([128, 128], BF, name=f"w_{h}_{qk}_{dt}",
                                       tag="w", bufs=24)
                        nc.sync.dma_start(wt[:], wqk[h, qk, dt])
                        wts.append(wt)
                    dest = work.tile([128, S], BF, name=f"qkT_{h}_{qk}",
                                     tag="qkT", bufs=4)
                    for tb in range(TB):
                        acc = psum.tile([128, 512], F32, name="acc_qk",
                                        tag="acc", bufs=2)
                        for dt in range(DT):
                            nc.tensor.matmul(
                                acc[:], wts[dt][:], xt_tiles[(dt, tb)][:],
                                start=(dt == 0), stop=(dt == DT - 1),
                            )
                        nc.scalar.activation(
                            dest[:, tb * 512:(tb + 1) * 512], acc[:],
                            mybir.ActivationFunctionType.Identity,
                            bias=bqk_sb[(h, qk)][:], scale=1.0,
                        )
                    dests[qk] = dest
                return dests[0], dests[1]

            # ---- attention for one head (q-blocks descending) + its AG ----
            def attention_head(h, qTh, kTh):
                for qb in (3, 2, 1, 0):
                    nk = 4 * qb + 4
                    y_ps = psum.tile([128, 512], F32, name="y_ps", tag="y")
                    esum = work.tile([128, 1024], BF, name="esum", tag="esum",
                                     bufs=2)
                    prev = None

                    def flush(prev_pair):
                        e, pr = prev_pair
                        for s_ in range(2):
                            kt = 2 * pr + s_
                            nc.tensor.matmul(
                                y_ps[:],
                                vt[kt][:, h * 128:(h + 1) * 128],
                                e[:, s_ * 512:(s_ + 1) * 512],
                                start=(kt == 0), stop=(kt == nk - 1),
                            )
                        if pr == 0:
                            nc.vector.tensor_copy(esum[:], e[:])
                        else:
                            nc.vector.tensor_tensor(esum[:], esum[:], e[:],
                                                    mybir.AluOpType.add)

                    for pr in range(nk // 2):
                        sc = psum.tile([128, 1024], F32, name="sc", tag="s",
                                       bufs=2)
                        for s_ in range(2):
                            kt = 2 * pr + s_
                            nc.tensor.matmul(
                                sc[:, s_ * 512:(s_ + 1) * 512],
                                kTh[:, kt * 128:(kt + 1) * 128],
                                qTh[:, qb * 512:(qb + 1) * 512],
                                start=True, stop=True,
                            )
                        e = work.tile([128, 1024], BF, name="expT", tag="expT",
                                      bufs=4)
                        nc.scalar.activation(
                            e[:], sc[:], mybir.ActivationFunctionType.Exp,
                            scale=SCALE,
                        )
                        pm = pr - (nk // 2 - 2)
                        if pm >= 0:
                            nc.vector.tensor_tensor(e[:], e[:],
                                                    pairmasks[pm][:],
                                                    mybir.AluOpType.mult)
                        if prev is not None:
                            flush(prev)
                        prev = (e, pr)
                    flush(prev)

                    esum_f = work.tile([128, 512], BF, name="esum_f",
                                       tag="esum_f", bufs=2)
                    nc.vector.tensor_tensor(esum_f[:], esum[:, 0:512],
                                            esum[:, 512:1024],
                                            mybir.AluOpType.add)
                    sum_ps = psum.tile([1, 512], F32, name="sum_ps", tag="y")
                    nc.tensor.matmul(sum_ps[:], ones[:], esum_f[:],
                                     start=True, stop=True)
                    recip = work.tile([1, 512], F32, name="recip", tag="recip",
                                      bufs=2)
                    nc.vector.reciprocal_approx_fast(recip[:], sum_ps[:])
                    rbc = work.tile([128, 512], F32, name="rbc", tag="rbc",
                                    bufs=2)
                    nc.gpsimd.partition_broadcast(rbc[:], recip[:], channels=128)
                    ynorm = work.tile([128, 512], BF, name="ynorm", tag="ynorm",
                                      bufs=3)
                    nc.vector.tensor_tensor(ynorm[:], y_ps[:], rbc[:],
                                            mybir.AluOpType.mult)
                    nc.sync.dma_start(
                        ag_in[h][:, qb * 512:(qb + 1) * 512], ynorm[:])
                nc.gpsimd.collective_compute(
                    "AllGather", mybir.AluOpType.bypass,
                    replica_groups=GROUPS,
                    ins=[ag_in[h].ap()], outs=[ag_out[h].ap()],
                )

            # ---- out-projection partial pass for head-chunk h ----
            wout_sb = {}

            def load_wout():
                for h in range(HLOC):
                    for i in range(4):
                        t = work.tile([128, CW], BF, name=f"wout{h}{i}",
                                      tag="p512", bufs=17)
                        nc.sync.dma_start(t[:], wout[h, i])
                        wout_sb[(h, i)] = t

            part = {}

            def outproj_pass(h):
                for tc_ in range(4):
                    ygt = []
                    for i in range(4):
                        t = work.tile([128, 512], BF, name=f"yg_{h}_{tc_}_{i}",
                                      tag="ygt", bufs=10)
                        nc.sync.dma_start(
                            t[:], ag_out[h][i * 128:(i + 1) * 128,
                                            tc_ * 512:(tc_ + 1) * 512])
                        ygt.append(t)
                    for j in range(4):
                        t = tc_ * 4 + j
                        acc = psum.tile([128, CW], F32, name="acc_o",
                                        tag="acc", bufs=2)
                        for i in range(4):
                            nc.tensor.matmul(
                                acc[:],
                                ygt[i][:, j * 128:(j + 1) * 128],
                                wout_sb[(h, i)][:],
                                start=(i == 0), stop=(i == 3),
                            )
                        if h == 0:
                            p = work.tile([128, CW], BF, name=f"part{t}",
                                          tag=f"part{t}", bufs=1)
                            part[t] = p
                            nc.vector.tensor_tensor(p[:], acc[:], bias_bc[:],
                                                    mybir.AluOpType.add)
                        elif h < HLOC - 1:
                            nc.vector.tensor_tensor(part[t][:], part[t][:],
                                                    acc[:],
                                                    mybir.AluOpType.add)
                        else:
                            osb = work.tile([128, CW], F32, name="osb",
                                            tag="osb", bufs=3)
                            nc.vector.tensor_tensor(osb[:], part[t][:], acc[:],
                                                    mybir.AluOpType.add)
                            nc.sync.dma_start(
                                out[t * 128:(t + 1) * 128, :], osb[:])

            # ---- head pipeline ----
            qk_tiles = qk_proj(0)
            for h in range(HLOC):
                next_qk = qk_proj(h + 1) if h + 1 < HLOC else None
                attention_head(h, *qk_tiles)
                if h == 0:
                    load_wout()
                if h > 0:
                    outproj_pass(h - 1)
                qk_tiles = next_qk
            outproj_pass(HLOC - 1)

    nc.compile()
    return nc


def _prep_inputs(x, w_qkv, b_qkv, w_out, b_out):
    """Host-side sharding/layout. Returns in_maps for the 8 cores."""
    bf16 = ml_dtypes.bfloat16
    x = np.asarray(x, dtype=np.float32)
    w_qkv = np.asarray(w_qkv, dtype=np.float32)
    b_qkv = np.asarray(b_qkv, dtype=np.float32)
    w_out = np.asarray(w_out, dtype=np.float32)
    b_out = np.asarray(b_out, dtype=np.float32)

    xT_b = [np.ascontiguousarray(x[b].T).astype(bf16) for b in range(B)]

    in_maps = []
    for c in range(8):
        b, g = c // 4, c % 4
        cols = slice(CW * g, CW * (g + 1))

        # wqk[h][0]=q, [1]=k tiles for global head 4g+h, [dt, 128, 128]
        wqk = np.empty((HLOC, 2, DT, 128, 128), np.float32)
        bqk = np.empty((HLOC, 2, 128, 1), np.float32)
        for h in range(HLOC):
            gh = 4 * g + h
            for qk in range(2):
                wcol = w_qkv[:, qk * D + 128 * gh: qk * D + 128 * (gh + 1)]
                wqk[h, qk] = wcol.reshape(DT, 128, 128)
                bqk[h, qk, :, 0] = b_qkv[qk * D + 128 * gh: qk * D + 128 * (gh + 1)]

        wv_ = w_qkv[:, 2 * D:3 * D][:, cols]
        bv_ = b_qkv[2 * D:3 * D][cols]

        # w_out rows permuted to the AG's rank-major order per head chunk
        wout_loc = w_out[:, cols]
        wout_t = np.empty((HLOC, 4, 128, CW), np.float32)
        for h in range(HLOC):
            for i in range(4):
                wout_t[h, i] = wout_loc[512 * i + 128 * h: 512 * i + 128 * (h + 1), :]

        in_maps.append({
            "xT": xT_b[b],
            "wqk": np.ascontiguousarray(wqk).astype(bf16),
            "wv": np.ascontiguousarray(wv_.reshape(DT, 128, CW)).astype(bf16),
            "bqk": np.ascontiguousarray(bqk),
            "bv": np.ascontiguousarray(bv_.reshape(1, CW)),
            "wout": np.ascontiguousarray(wout_t).astype(bf16),
            "bout": np.ascontiguousarray(b_out[cols].reshape(1, CW)),
        })
    return in_maps


def kernel(x, w_qkv, b_qkv, w_out, b_out, _trace=False, _trace_kwargs=None):
    from concourse.bass_utils import run_bass_kernel_spmd

    if "nc" not in _cache:
        _cache["nc"] = _build()
    nc = _cache["nc"]

    in_maps = _prep_inputs(x, w_qkv, b_qkv, w_out, b_out)
    res = run_bass_kernel_spmd(
        nc, in_maps, core_ids=list(range(8)),
        trace=_trace, **(_trace_kwargs or {}),
    )

    out = np.empty((B, S, D), dtype=np.float32)
    for c in range(8):
        b, g = c // 4, c % 4
        out[b][:, CW * g:CW * (g + 1)] = res.results[c]["out"]
    kernel.last_result = res
    return out


# revision 21
# speedup vs baseline: 1.1627x; 1.0410x over previous
"""Causal self-attention (B=2, S=2048, D=2048, H=16) on 8 TRN2 NeuronCores.

Sharding: 2 batches x 4 head-groups.  Core c handles batch c//4 and heads
[4*(c%4) .. 4*(c%4)+3]; each core produces output columns [512*(c%4) ...].

Per core, head-pipelined so the AllGathers hide under compute:
  v projection (all 4 heads), then for each local head h:
    q/k projection (transposed layout) -> causal attention over 4 q-blocks
    -> AllGather (groups of 4) of that head's normalized yT (bf16)
    -> out-projection partial pass for head-chunk h-1 (one AG behind)
The out projection accumulates head-chunk partials (bf16 SBUF) with w_out
rows host-permuted to match the AG's rank-major row order; only the last
head's AG + partial pass is exposed comm.

Softmax uses exp without max subtraction (logits are O(8) here); denominators
are accumulated on DVE over k-tile pairs then reduced across partitions with
a single ones-matmul, inverted with reciprocal_approx_fast.

Compute is bf16 with fp32 PSUM accumulation; measured l2 rel err vs the fp32
reference is ~5.5e-3.
"""

import numpy as np
import ml_dtypes

B, S, D = 2, 2048, 2048
H, HD = 16, 128
HLOC = 4           # heads per core
CW = HLOC * HD     # 512: per-core v width and out-column width
QB = 4             # q blocks of 512
DT = 16            # d tiles of 128
TB = 4             # token blocks of 512
SCALE = 1.0 / float(np.sqrt(HD))
GROUPS = [[0, 1, 2, 3], [4, 5, 6, 7]]

_cache = {}


def _build():
    import concourse.tile as tile
    import concourse.mybir as mybir
    from concourse import bacc

    BF = mybir.dt.bfloat16
    F32 = mybir.dt.float32

    nc = bacc.Bacc("TRN2", target_bir_lowering=False, debug=False, num_devices=8)

    # Inputs (per-core shards, host-prepared)
    xT = nc.dram_tensor("xT", [D, S], BF, kind="ExternalInput")          # x[batch].T
    wqk = nc.dram_tensor("wqk", [HLOC, 2, DT, 128, 128], BF, kind="ExternalInput")
    wv = nc.dram_tensor("wv", [DT, 128, CW], BF, kind="ExternalInput")
    bqk = nc.dram_tensor("bqk", [HLOC, 2, 128, 1], F32, kind="ExternalInput")
    bv = nc.dram_tensor("bv", [1, CW], F32, kind="ExternalInput")
    # w_out rows permuted: wout[h][i] = w_out[512*i + 128*h : +128, cols]
    wout = nc.dram_tensor("wout", [HLOC, 4, 128, CW], BF, kind="ExternalInput")
    bout = nc.dram_tensor("bout", [1, CW], F32, kind="ExternalInput")
    out = nc.dram_tensor("out", [S, CW], F32, kind="ExternalOutput")

    ag_in = [nc.dram_tensor(f"ag_in{h}", [128, S], BF, kind="Internal")
             for h in range(HLOC)]
    ag_out = [nc.dram_tensor(f"ag_out{h}", [512, S], BF, kind="Internal")
              for h in range(HLOC)]

    with tile.TileContext(nc) as tc:
        with (
            tc.tile_pool(name="const", bufs=1) as constp,
            tc.tile_pool(name="pers", bufs=1) as pers,
            tc.tile_pool(name="work", bufs=2) as work,
            tc.tile_pool(name="psum", bufs=2, space="PSUM") as psum,
        ):
            # ---- constants ----
            ones = constp.tile([128, 1], BF, name="ones")
            nc.gpsimd.memset(ones[:], 1.0)

            # Pair masks for the 4 diagonal k-subtiles, packed two subtiles
            # wide: pairmask[m][:, 512*sub + qq] keeps where
            # qq >= kk + 128*(2m+sub).
            pairmasks = []
            for pm in range(2):
                m = constp.tile([128, 1024], BF, name=f"pmask{pm}",
                                tag=f"pmask{pm}")
                nc.gpsimd.memset(m[:], 1.0)
                for sub in range(2):
                    nc.gpsimd.affine_select(
                        out=m[:, sub * 512:(sub + 1) * 512],
                        in_=m[:, sub * 512:(sub + 1) * 512],
                        compare_op=mybir.AluOpType.is_ge, fill=0.0,
                        base=-128 * (2 * pm + sub), channel_multiplier=-1,
                        pattern=[[1, 512]],
                    )
                pairmasks.append(m)

            bout_sb = constp.tile([1, CW], F32, name="bout_sb")
            nc.sync.dma_start(bout_sb[:], bout[:])
            bias_bc = constp.tile([128, CW], F32, name="bias_bc")
            nc.gpsimd.partition_broadcast(bias_bc[:], bout_sb[:], channels=128)

            bv_sb = constp.tile([1, CW], F32, name="bv_sb")
            nc.sync.dma_start(bv_sb[:], bv[:])
            vbias_bc = constp.tile([128, CW], F32, name="vbias_bc")
            nc.gpsimd.partition_broadcast(vbias_bc[:], bv_sb[:], channels=128)

            bqk_sb = {}
            for h in range(HLOC):
                for qk in range(2):
                    t = constp.tile([128, 1], F32, name=f"bqk{h}{qk}",
                                    tag=f"bqk{h}{qk}")
                    nc.sync.dma_start(t[:], bqk[h, qk])
                    bqk_sb[(h, qk)] = t

            # ---- persistent v tiles ----
            vt = [pers.tile([128, CW], BF, name=f"v{t}", tag=f"v{t}")
                  for t in range(16)]

            # ---- loads: wv panels first, then xT tb-major ----
            wv_sb = []
            for dt in range(DT):
                wvp = work.tile([128, CW], BF, name=f"wvp{dt}", tag="p512",
                                bufs=17)
                nc.sync.dma_start(wvp[:], wv[dt])
                wv_sb.append(wvp)
            xt_tiles = {}
            for tb in range(TB):
                for dt in range(DT):
                    t = work.tile([128, 512], BF, name=f"xt_{dt}_{tb}",
                                  tag="xT", bufs=68)
                    nc.sync.dma_start(
                        t[:], xT[dt * 128:(dt + 1) * 128, tb * 512:(tb + 1) * 512])
                    xt_tiles[(dt, tb)] = t

            # ---- v projection: v[t] = x @ wv  ([tok, vcol], xT stationary) ----
            for t in range(16):
                tb, j = t // 4, t % 4
                acc = psum.tile([128, CW], F32, name="acc_v", tag="acc", bufs=2)
                for dt in range(DT):
                    nc.tensor.matmul(
                        acc[:],
                        xt_tiles[(dt, tb)][:, j * 128:(j + 1) * 128],
                        wv_sb[dt][:],
                        start=(dt == 0), stop=(dt == DT - 1),
                    )
                nc.vector.tensor_tensor(vt[t][:], acc[:], vbias_bc[:],
                                        mybir.AluOpType.add)

            # ---- per-head q/k projection ([col, tok] transposed) ----
            def qk_proj(h):
                dests = {}
                for qk in range(2):
                    wts = []
                    for dt in range(DT):
                        wt = work.tile([128, 128], BF, name=f"w_{h}_{qk}_{dt}",
                                       tag="w", bufs=24)
                        nc.sync.dma_start(wt[:], wqk[h, qk, dt])
                        wts.append(wt)
                    dest = work.tile([128, S], BF, name=f"qkT_{h}_{qk}",
                                     tag="qkT", bufs=4)
                    for tb in range(TB):
                        acc = psum.tile([128, 512], F32, name="acc_qk",
                                        tag="acc", bufs=2)
                        for dt in range(DT):
                            nc.tensor.matmul(
                                acc[:], wts[dt][:], xt_tiles[(dt, tb)][:],
                                start=(dt == 0), stop=(dt == DT - 1),
                            )
                        nc.scalar.activation(
                            dest[:, tb * 512:(tb + 1) * 512], acc[:],
                            mybir.ActivationFunctionType.Identity,
                            bias=bqk_sb[(h, qk)][:], scale=1.0,
                        )
                    dests[qk] = dest
                return dests[0], dests[1]

            # ---- attention for one head (q-blocks descending) + its AG ----
            def attention_head(h, qTh, kTh):
                for qb in (3, 2, 1, 0):
                    nk = 4 * qb + 4
                    y_ps = psum.tile([128, 512], F32, name="y_ps", tag="y")
                    esum = work.tile([128, 1024], BF, name="esum", tag="esum",
                                     bufs=2)
                    prev = None

                    def flush(prev_pair):
                        e, pr = prev_pair
                        for s_ in range(2):
                            kt = 2 * pr + s_
                            nc.tensor.matmul(
                                y_ps[:],
                                vt[kt][:, h * 128:(h + 1) * 128],
                                e[:, s_ * 512:(s_ + 1) * 512],
                                start=(kt == 0), stop=(kt == nk - 1),
                            )
                        if pr == 0:
                            nc.vector.tensor_copy(esum[:], e[:])
                        else:
                            nc.vector.tensor_tensor(esum[:], esum[:], e[:],
                                                    mybir.AluOpType.add)

                    for pr in range(nk // 2):
                        sc = psum.tile([128, 1024], F32, name="sc", tag="s",
                                       bufs=2)
                        for s_ in range(2):
                            kt = 2 * pr + s_
                            nc.tensor.matmul(
                                sc[:, s_ * 512:(s_ + 1) * 512],
                                kTh[:, kt * 128:(kt + 1) * 128],
                                qTh[:, qb * 512:(qb + 1) * 512],
                                start=True, stop=True,
                            )
                        e = work.tile([128, 1024], BF, name="expT", tag="expT",
                                      bufs=4)
                        nc.scalar.activation(
                            e[:], sc[:], mybir.ActivationFunctionType.Exp,
                            scale=SCALE,
                        )
                        pm = pr - (nk // 2 - 2)
                        if pm >= 0:
                            nc.vector.tensor_tensor(e[:], e[:],
                                                    pairmasks[pm][:],
                                                    mybir.AluOpType.mult)
                        if prev is not None:
                            flush(prev)
                        prev = (e, pr)
                    flush(prev)

                    esum_f = work.tile([128, 512], BF, name="esum_f",
                                       tag="esum_f", bufs=2)
                    nc.vector.tensor_tensor(esum_f[:], esum[:, 0:512],
                                            esum[:, 512:1024],
                                            mybir.AluOpType.add)
                    sum_ps = psum.tile([1, 512], F32, name="sum_ps", tag="y")
                    nc.tensor.matmul(sum_ps[:], ones[:], esum_f[:],
                                     start=True, stop=True)
                    recip = work.tile([1, 512], F32, name="recip", tag="recip",
                                      bufs=2)
                    nc.vector.reciprocal_approx_fast(recip[:], sum_ps[:])
                    rbc = work.tile([128, 512], F32, name="rbc", tag="rbc",
                                    bufs=2)
                    nc.gpsimd.partition_broadcast(rbc[:], recip[:], channels=128)
                    ynorm = work.tile([128, 512], BF, name="ynorm", tag="ynorm",
                                      bufs=3)
                    nc.vector.tensor_tensor(ynorm[:], y_ps[:], rbc[:],
                                            mybir.AluOpType.mult)
                    nc.sync.dma_start(
                        ag_in[h][:, qb * 512:(qb + 1) * 512], ynorm[:])
                nc.gpsimd.collective_compute(
                    "AllGather", mybir.AluOpType.bypass,
                    replica_groups=GROUPS,
                    ins=[ag_in[h].ap()], outs=[ag_out[h].ap()],
                )

            # ---- out-projection partial pass for head-chunk h ----
            wout_sb = {}

            def load_wout():
                for h in range(HLOC):
                    for i in range(4):
                        t = work.tile([128, CW], BF, name=f"wout{h}{i}",
                                      tag="p512", bufs=17)
                        nc.sync.dma_start(t[:], wout[h, i])
                        wout_sb[(h, i)] = t

            part = {}

            def outproj_pass(h):
                for tc_ in range(4):
                    ygt = []
                    for i in range(4):
                        t = work.tile([128, 512], BF, name=f"yg_{h}_{tc_}_{i}",
                                      tag="ygt", bufs=10)
                        nc.sync.dma_start(
                            t[:], ag_out[h][i * 128:(i + 1) * 128,
                                            tc_ * 512:(tc_ + 1) * 512])
                        ygt.append(t)
                    for j in range(4):
                        t = tc_ * 4 + j
                        acc = psum.tile([128, CW], F32, name="acc_o",
                                        tag="acc", bufs=2)
                        for i in range(4):
                            nc.tensor.matmul(
                                acc[:],
                                ygt[i][:, j * 128:(j + 1) * 128],
                                wout_sb[(h, i)][:],
                                start=(i == 0), stop=(i == 3),
                            )
                        if h == 0:
                            p = work.tile([128, CW], BF, name=f"part{t}",
                                          tag=f"part{t}", bufs=1)
                            part[t] = p
                            nc.vector.tensor_tensor(p[:], acc[:], bias_bc[:],
                                                    mybir.AluOpType.add)
                        elif h < HLOC - 1:
                            nc.vector.tensor_tensor(part[t][:], part[t][:],
                                                    acc[:],
                                                    mybir.AluOpType.add)
                        else:
                            osb = work.tile([128, CW], F32, name="osb",
                                            tag="osb", bufs=3)
                            nc.vector.tensor_tensor(osb[:], part[t][:], acc[:],
                                                    mybir.AluOpType.add)
                            nc.sync.dma_start(
                                out[t * 128:(t + 1) * 128, :], osb[:])

            # ---- head pipeline: attention first (AG trigger asap), then
            # next head's projection, then the pass for the landed AG ----
            qk_tiles = qk_proj(0)
            load_wout()
            for h in range(HLOC):
                attention_head(h, *qk_tiles)
                qk_tiles = qk_proj(h + 1) if h + 1 < HLOC else None
                if h > 0:
                    outproj_pass(h - 1)
            outproj_pass(HLOC - 1)

    nc.compile()
    return nc


def _prep_inputs(x, w_qkv, b_qkv, w_out, b_out):
    """Host-side sharding/layout. Returns in_maps for the 8 cores."""
    bf16 = ml_dtypes.bfloat16
    x = np.asarray(x, dtype=np.float32)
    w_qkv = np.asarray(w_qkv, dtype=np.float32)
    b_qkv = np.asarray(b_qkv, dtype=np.float32)
    w_out = np.asarray(w_out, dtype=np.float32)
    b_out = np.asarray(b_out, dtype=np.float32)

    xT_b = [np.ascontiguousarray(x[b].T).astype(bf16) for b in range(B)]

    in_maps = []
    for c in range(8):
        b, g = c // 4, c % 4
        cols = slice(CW * g, CW * (g + 1))

        # wqk[h][0]=q, [1]=k tiles for global head 4g+h, [dt, 128, 128]
        wqk = np.empty((HLOC, 2, DT, 128, 128), np.float32)
        bqk = np.empty((HLOC, 2, 128, 1), np.float32)
        for h in range(HLOC):
            gh = 4 * g + h
            for qk in range(2):
                wcol = w_qkv[:, qk * D + 128 * gh: qk * D + 128 * (gh + 1)]
                wqk[h, qk] = wcol.reshape(DT, 128, 128)
                bqk[h, qk, :, 0] = b_qkv[qk * D + 128 * gh: qk * D + 128 * (gh + 1)]

        wv_ = w_qkv[:, 2 * D:3 * D][:, cols]
        bv_ = b_qkv[2 * D:3 * D][cols]

        # w_out rows permuted to the AG's rank-major order per head chunk
        wout_loc = w_out[:, cols]
        wout_t = np.empty((HLOC, 4, 128, CW), np.float32)
        for h in range(HLOC):
            for i in range(4):
                wout_t[h, i] = wout_loc[512 * i + 128 * h: 512 * i + 128 * (h + 1), :]

        in_maps.append({
            "xT": xT_b[b],
            "wqk": np.ascontiguousarray(wqk).astype(bf16),
            "wv": np.ascontiguousarray(wv_.reshape(DT, 128, CW)).astype(bf16),
            "bqk": np.ascontiguousarray(bqk),
            "bv": np.ascontiguousarray(bv_.reshape(1, CW)),
            "wout": np.ascontiguousarray(wout_t).astype(bf16),
            "bout": np.ascontiguousarray(b_out[cols].reshape(1, CW)),
        })
    return in_maps


def kernel(x, w_qkv, b_qkv, w_out, b_out, _trace=False, _trace_kwargs=None):
    from concourse.bass_utils import run_bass_kernel_spmd

    if "nc" not in _cache:
        _cache["nc"] = _build()
    nc = _cache["nc"]

    in_maps = _prep_inputs(x, w_qkv, b_qkv, w_out, b_out)
    res = run_bass_kernel_spmd(
        nc, in_maps, core_ids=list(range(8)),
        trace=_trace, **(_trace_kwargs or {}),
    )

    out = np.empty((B, S, D), dtype=np.float32)
    for c in range(8):
        b, g = c // 4, c % 4
        out[b][:, CW * g:CW * (g + 1)] = res.results[c]["out"]
    kernel.last_result = res
    return out


# revision 22
# speedup vs baseline: 1.2146x; 1.0446x over previous
"""Causal self-attention (B=2, S=2048, D=2048, H=16) on 8 TRN2 NeuronCores.

Sharding: 2 batches x 4 head-groups.  Core c handles batch c//4 and heads
[4*(c%4) .. 4*(c%4)+3]; each core produces output columns [512*(c%4) ...].

Per core, head-pipelined so the AllGathers hide under compute:
  v projection (all 4 heads), then for each local head h:
    q/k projection (transposed layout) -> causal attention over 4 q-blocks
    -> AllGather (groups of 4) of that head's normalized yT (bf16)
    -> out-projection partial pass for head-chunk h-1 (one AG behind)
The out projection accumulates head-chunk partials (bf16 SBUF) with w_out
rows host-permuted to match the AG's rank-major row order; only the last
head's AG + partial pass is exposed comm.

Softmax uses exp without max subtraction (logits are O(8) here); denominators
are accumulated on DVE over k-tile pairs then reduced across partitions with
a single ones-matmul, inverted with reciprocal_approx_fast.

Compute is bf16 with fp32 PSUM accumulation; measured l2 rel err vs the fp32
reference is ~5.5e-3.
"""

import numpy as np
import ml_dtypes

B, S, D = 2, 2048, 2048
H, HD = 16, 128
HLOC = 4           # heads per core
CW = HLOC * HD     # 512: per-core v width and out-column width
QB = 4             # q blocks of 512
DT = 16            # d tiles of 128
TB = 4             # token blocks of 512
SCALE = 1.0 / float(np.sqrt(HD))
GROUPS = [[0, 1, 2, 3], [4, 5, 6, 7]]

_cache = {}


def _build():
    import concourse.tile as tile
    import concourse.mybir as mybir
    from concourse import bacc

    BF = mybir.dt.bfloat16
    F32 = mybir.dt.float32

    nc = bacc.Bacc("TRN2", target_bir_lowering=False, debug=False, num_devices=8)

    # Inputs (per-core shards, host-prepared)
    xT = nc.dram_tensor("xT", [D, S], BF, kind="ExternalInput")          # x[batch].T
    wqk = nc.dram_tensor("wqk", [HLOC, 2, DT, 128, 128], BF, kind="ExternalInput")
    wv = nc.dram_tensor("wv", [DT, 128, CW], BF, kind="ExternalInput")
    bqk = nc.dram_tensor("bqk", [HLOC, 2, 128, 1], F32, kind="ExternalInput")
    bv = nc.dram_tensor("bv", [1, CW], F32, kind="ExternalInput")
    # w_out rows permuted: wout[h][i] = w_out[512*i + 128*h : +128, cols]
    wout = nc.dram_tensor("wout", [HLOC, 4, 128, CW], BF, kind="ExternalInput")
    bout = nc.dram_tensor("bout", [1, CW], F32, kind="ExternalInput")
    out = nc.dram_tensor("out", [S, CW], F32, kind="ExternalOutput")

    ag_in = [nc.dram_tensor(f"ag_in{h}", [128, S], BF, kind="Internal")
             for h in range(HLOC)]
    ag_out = [nc.dram_tensor(f"ag_out{h}", [512, S], BF, kind="Internal")
              for h in range(HLOC)]

    with tile.TileContext(nc) as tc:
        with (
            tc.tile_pool(name="const", bufs=1) as constp,
            tc.tile_pool(name="pers", bufs=1) as pers,
            tc.tile_pool(name="work", bufs=2) as work,
            tc.tile_pool(name="psum", bufs=2, space="PSUM") as psum,
        ):
            # ---- constants ----
            ones = constp.tile([128, 1], BF, name="ones")
            nc.gpsimd.memset(ones[:], 1.0)

            # Pair masks for the 4 diagonal k-subtiles, packed two subtiles
            # wide: pairmask[m][:, 512*sub + qq] keeps where
            # qq >= kk + 128*(2m+sub).
            pairmasks = []
            for pm in range(2):
                m = constp.tile([128, 1024], BF, name=f"pmask{pm}",
                                tag=f"pmask{pm}")
                nc.gpsimd.memset(m[:], 1.0)
                for sub in range(2):
                    nc.gpsimd.affine_select(
                        out=m[:, sub * 512:(sub + 1) * 512],
                        in_=m[:, sub * 512:(sub + 1) * 512],
                        compare_op=mybir.AluOpType.is_ge, fill=0.0,
                        base=-128 * (2 * pm + sub), channel_multiplier=-1,
                        pattern=[[1, 512]],
                    )
                pairmasks.append(m)

            bout_sb = constp.tile([1, CW], F32, name="bout_sb")
            nc.sync.dma_start(bout_sb[:], bout[:])
            bias_bc = constp.tile([128, CW], F32, name="bias_bc")
            nc.gpsimd.partition_broadcast(bias_bc[:], bout_sb[:], channels=128)

            bv_sb = constp.tile([1, CW], F32, name="bv_sb")
            nc.sync.dma_start(bv_sb[:], bv[:])
            vbias_bc = constp.tile([128, CW], F32, name="vbias_bc")
            nc.gpsimd.partition_broadcast(vbias_bc[:], bv_sb[:], channels=128)

            bqk_sb = {}
            for h in range(HLOC):
                for qk in range(2):
                    t = constp.tile([128, 1], F32, name=f"bqk{h}{qk}",
                                    tag=f"bqk{h}{qk}")
                    nc.sync.dma_start(t[:], bqk[h, qk])
                    bqk_sb[(h, qk)] = t

            # ---- persistent v tiles ----
            vt = [pers.tile([128, CW], BF, name=f"v{t}", tag=f"v{t}")
                  for t in range(16)]

            # ---- loads: wv panels first, then xT tb-major ----
            wv_sb = []
            for dt in range(DT):
                wvp = work.tile([128, CW], BF, name=f"wvp{dt}", tag="p512",
                                bufs=17)
                nc.sync.dma_start(wvp[:], wv[dt])
                wv_sb.append(wvp)
            xt_tiles = {}
            for tb in range(TB):
                for dt in range(DT):
                    t = work.tile([128, 512], BF, name=f"xt_{dt}_{tb}",
                                  tag="xT", bufs=68)
                    nc.sync.dma_start(
                        t[:], xT[dt * 128:(dt + 1) * 128, tb * 512:(tb + 1) * 512])
                    xt_tiles[(dt, tb)] = t

            # ---- v projection: v[t] = x @ wv  ([tok, vcol], xT stationary) ----
            for t in range(16):
                tb, j = t // 4, t % 4
                acc = psum.tile([128, CW], F32, name="acc_v", tag="acc", bufs=2)
                for dt in range(DT):
                    nc.tensor.matmul(
                        acc[:],
                        xt_tiles[(dt, tb)][:, j * 128:(j + 1) * 128],
                        wv_sb[dt][:],
                        start=(dt == 0), stop=(dt == DT - 1),
                    )
                nc.vector.tensor_tensor(vt[t][:], acc[:], vbias_bc[:],
                                        mybir.AluOpType.add)

            # ---- per-head q/k projection ([col, tok] transposed) ----
            def qk_proj(h):
                dests = {}
                for qk in range(2):
                    wts = []
                    for dt in range(DT):
                        wt = work.tile([128, 128], BF, name=f"w_{h}_{qk}_{dt}",
                                       tag="w", bufs=24)
                        nc.sync.dma_start(wt[:], wqk[h, qk, dt])
                        wts.append(wt)
                    dest = work.tile([128, S], BF, name=f"qkT_{h}_{qk}",
                                     tag="qkT", bufs=4)
                    for tb in range(TB):
                        acc = psum.tile([128, 512], F32, name="acc_qk",
                                        tag="acc", bufs=2)
                        for dt in range(DT):
                            nc.tensor.matmul(
                                acc[:], wts[dt][:], xt_tiles[(dt, tb)][:],
                                start=(dt == 0), stop=(dt == DT - 1),
                            )
                        nc.scalar.activation(
                            dest[:, tb * 512:(tb + 1) * 512], acc[:],
                            mybir.ActivationFunctionType.Identity,
                            bias=bqk_sb[(h, qk)][:], scale=1.0,
                        )
                    dests[qk] = dest
                return dests[0], dests[1]

            # ---- attention for one head (q-blocks descending) + its AG ----
            def attention_head(h, qTh, kTh):
                for qb in (3, 2, 1, 0):
                    nk = 4 * qb + 4
                    y_ps = psum.tile([128, 512], F32, name="y_ps", tag="y")
                    esum = work.tile([128, 1024], BF, name="esum", tag="esum",
                                     bufs=2)
                    prev = None

                    def flush(prev_pair):
                        e, pr = prev_pair
                        for s_ in range(2):
                            kt = 2 * pr + s_
                            nc.tensor.matmul(
                                y_ps[:],
                                vt[kt][:, h * 128:(h + 1) * 128],
                                e[:, s_ * 512:(s_ + 1) * 512],
                                start=(kt == 0), stop=(kt == nk - 1),
                            )
                        if pr == 0:
                            nc.vector.tensor_copy(esum[:], e[:])
                        else:
                            nc.vector.tensor_tensor(esum[:], esum[:], e[:],
                                                    mybir.AluOpType.add)

                    for pr in range(nk // 2):
                        sc = psum.tile([128, 1024], F32, name="sc", tag="s",
                                       bufs=2)
                        for s_ in range(2):
                            kt = 2 * pr + s_
                            nc.tensor.matmul(
                                sc[:, s_ * 512:(s_ + 1) * 512],
                                kTh[:, kt * 128:(kt + 1) * 128],
                                qTh[:, qb * 512:(qb + 1) * 512],
                                start=True, stop=True,
                            )
                        e = work.tile([128, 1024], BF, name="expT", tag="expT",
                                      bufs=4)
                        nc.scalar.activation(
                            e[:], sc[:], mybir.ActivationFunctionType.Exp,
                            scale=SCALE,
                        )
                        pm = pr - (nk // 2 - 2)
                        if pm >= 0:
                            nc.vector.tensor_tensor(e[:], e[:],
                                                    pairmasks[pm][:],
                                                    mybir.AluOpType.mult)
                        if prev is not None:
                            flush(prev)
                        prev = (e, pr)
                    flush(prev)

                    esum_f = work.tile([128, 512], BF, name="esum_f",
                                       tag="esum_f", bufs=2)
                    nc.vector.tensor_tensor(esum_f[:], esum[:, 0:512],
                                            esum[:, 512:1024],
                                            mybir.AluOpType.add)
                    sum_ps = psum.tile([1, 512], F32, name="sum_ps", tag="y")
                    nc.tensor.matmul(sum_ps[:], ones[:], esum_f[:],
                                     start=True, stop=True)
                    recip = work.tile([1, 512], F32, name="recip", tag="recip",
                                      bufs=2)
                    nc.vector.reciprocal_approx_fast(recip[:], sum_ps[:])
                    rbc = work.tile([128, 512], F32, name="rbc", tag="rbc",
                                    bufs=2)
                    nc.gpsimd.partition_broadcast(rbc[:], recip[:], channels=128)
                    ynorm = work.tile([128, 512], BF, name="ynorm", tag="ynorm",
                                      bufs=3)
                    nc.vector.tensor_tensor(ynorm[:], y_ps[:], rbc[:],
                                            mybir.AluOpType.mult)
                    nc.sync.dma_start(
                        ag_in[h][:, qb * 512:(qb + 1) * 512], ynorm[:])
                nc.gpsimd.collective_compute(
                    "AllGather", mybir.AluOpType.bypass,
                    replica_groups=GROUPS,
                    ins=[ag_in[h].ap()], outs=[ag_out[h].ap()],
                )

            # ---- out-projection partial pass for head-chunk h ----
            wout_sb = {}

            def load_wout():
                for h in range(HLOC):
                    for i in range(4):
                        t = work.tile([128, CW], BF, name=f"wout{h}{i}",
                                      tag="p512", bufs=17)
                        nc.sync.dma_start(t[:], wout[h, i])
                        wout_sb[(h, i)] = t

            part = {}

            def outproj_pass(h):
                for tc_ in range(4):
                    ygt = []
                    for i in range(4):
                        t = work.tile([128, 512], BF, name=f"yg_{h}_{tc_}_{i}",
                                      tag="ygt", bufs=10)
                        nc.sync.dma_start(
                            t[:], ag_out[h][i * 128:(i + 1) * 128,
                                            tc_ * 512:(tc_ + 1) * 512])
                        ygt.append(t)
                    for j in range(4):
                        t = tc_ * 4 + j
                        acc = psum.tile([128, CW], F32, name="acc_o",
                                        tag="acc", bufs=2)
                        for i in range(4):
                            nc.tensor.matmul(
                                acc[:],
                                ygt[i][:, j * 128:(j + 1) * 128],
                                wout_sb[(h, i)][:],
                                start=(i == 0), stop=(i == 3),
                            )
                        if h == 0:
                            p = work.tile([128, CW], BF, name=f"part{t}",
                                          tag=f"part{t}", bufs=1)
                            part[t] = p
                            nc.vector.tensor_tensor(p[:], acc[:], bias_bc[:],
                                                    mybir.AluOpType.add)
                        elif h < HLOC - 1:
                            nc.vector.tensor_tensor(part[t][:], part[t][:],
                                                    acc[:],
                                                    mybir.AluOpType.add)
                        else:
                            osb = work.tile([128, CW], F32, name="osb",
                                            tag="osb", bufs=3)
                            nc.vector.tensor_tensor(osb[:], part[t][:], acc[:],
                                                    mybir.AluOpType.add)
                            nc.sync.dma_start(
                                out[t * 128:(t + 1) * 128, :], osb[:])

            # ---- head pipeline: attention first (AG trigger asap), then
            # next head's projection, then the pass for the landed AG ----
            qk_tiles = qk_proj(0)
            load_wout()
            for h in range(HLOC):
                attention_head(h, *qk_tiles)
                qk_tiles = qk_proj(h + 1) if h + 1 < HLOC else None
                if h > 1:
                    outproj_pass(h - 2)
            outproj_pass(HLOC - 2)
            outproj_pass(HLOC - 1)

    nc.compile()
    return nc


def _prep_inputs(x, w_qkv, b_qkv, w_out, b_out):
    """Host-side sharding/layout. Returns in_maps for the 8 cores."""
    bf16 = ml_dtypes.bfloat16
    x = np.asarray(x, dtype=np.float32)
    w_qkv = np.asarray(w_qkv, dtype=np.float32)
    b_qkv = np.asarray(b_qkv, dtype=np.float32)
    w_out = np.asarray(w_out, dtype=np.float32)
    b_out = np.asarray(b_out, dtype=np.float32)

    xT_b = [np.ascontiguousarray(x[b].T).astype(bf16) for b in range(B)]

    in_maps = []
    for c in range(8):
        b, g = c // 4, c % 4
        cols = slice(CW * g, CW * (g + 1))

        # wqk[h][0]=q, [1]=k tiles for global head 4g+h, [dt, 128, 128]
        wqk = np.empty((HLOC, 2, DT, 128, 128), np.float32)
        bqk = np.empty((HLOC, 2, 128, 1), np.float32)
        for h in range(HLOC):
            gh = 4 * g + h
            for qk in range(2):
                wcol = w_qkv[:, qk * D + 128 * gh: qk * D + 128 * (gh + 1)]
                wqk[h, qk] = wcol.reshape(DT, 128, 128)
                bqk[h, qk, :, 0] = b_qkv[qk * D + 128 * gh: qk * D + 128 * (gh + 1)]

        wv_ = w_qkv[:, 2 * D:3 * D][:, cols]
        bv_ = b_qkv[2 * D:3 * D][cols]

        # w_out rows permuted to the AG's rank-major order per head chunk
        wout_loc = w_out[:, cols]
        wout_t = np.empty((HLOC, 4, 128, CW), np.float32)
        for h in range(HLOC):
            for i in range(4):
                wout_t[h, i] = wout_loc[512 * i + 128 * h: 512 * i + 128 * (h + 1), :]

        in_maps.append({
            "xT": xT_b[b],
            "wqk": np.ascontiguousarray(wqk).astype(bf16),
            "wv": np.ascontiguousarray(wv_.reshape(DT, 128, CW)).astype(bf16),
            "bqk": np.ascontiguousarray(bqk),
            "bv": np.ascontiguousarray(bv_.reshape(1, CW)),
            "wout": np.ascontiguousarray(wout_t).astype(bf16),
            "bout": np.ascontiguousarray(b_out[cols].reshape(1, CW)),
        })
    return in_maps


def kernel(x, w_qkv, b_qkv, w_out, b_out, _trace=False, _trace_kwargs=None):
    from concourse.bass_utils import run_bass_kernel_spmd

    if "nc" not in _cache:
        _cache["nc"] = _build()
    nc = _cache["nc"]

    in_maps = _prep_inputs(x, w_qkv, b_qkv, w_out, b_out)
    res = run_bass_kernel_spmd(
        nc, in_maps, core_ids=list(range(8)),
        trace=_trace, **(_trace_kwargs or {}),
    )

    out = np.empty((B, S, D), dtype=np.float32)
    for c in range(8):
        b, g = c // 4, c % 4
        out[b][:, CW * g:CW * (g + 1)] = res.results[c]["out"]
    kernel.last_result = res
    return out


# revision 23
# speedup vs baseline: 1.2198x; 1.0043x over previous
"""Causal self-attention (B=2, S=2048, D=2048, H=16) on 8 TRN2 NeuronCores.

Sharding: 2 batches x 4 head-groups.  Core c handles batch c//4 and heads
[4*(c%4) .. 4*(c%4)+3]; each core produces output columns [512*(c%4) ...].

Per core, head-pipelined so the AllGathers hide under compute:
  v projection (all 4 heads), then for each local head h:
    q/k projection (transposed layout) -> causal attention over 4 q-blocks
    -> AllGather (groups of 4) of that head's normalized yT (bf16)
    -> out-projection partial pass for head-chunk h-1 (one AG behind)
The out projection accumulates head-chunk partials (bf16 SBUF) with w_out
rows host-permuted to match the AG's rank-major row order; only the last
head's AG + partial pass is exposed comm.

Softmax uses exp without max subtraction (logits are O(8) here); denominators
are accumulated on DVE over k-tile pairs then reduced across partitions with
a single ones-matmul, inverted with reciprocal_approx_fast.

Compute is bf16 with fp32 PSUM accumulation; measured l2 rel err vs the fp32
reference is ~5.5e-3.
"""

import numpy as np
import ml_dtypes

B, S, D = 2, 2048, 2048
H, HD = 16, 128
HLOC = 4           # heads per core
CW = HLOC * HD     # 512: per-core v width and out-column width
QB = 4             # q blocks of 512
DT = 16            # d tiles of 128
TB = 4             # token blocks of 512
SCALE = 1.0 / float(np.sqrt(HD))
GROUPS = [[0, 1, 2, 3], [4, 5, 6, 7]]

_cache = {}


def _build():
    import concourse.tile as tile
    import concourse.mybir as mybir
    from concourse import bacc

    BF = mybir.dt.bfloat16
    F32 = mybir.dt.float32

    nc = bacc.Bacc("TRN2", target_bir_lowering=False, debug=False, num_devices=8)

    # Inputs (per-core shards, host-prepared)
    xT = nc.dram_tensor("xT", [D, S], BF, kind="ExternalInput")          # x[batch].T
    wqk = nc.dram_tensor("wqk", [HLOC, 2, DT, 128, 128], BF, kind="ExternalInput")
    wv = nc.dram_tensor("wv", [DT, 128, CW], BF, kind="ExternalInput")
    bqk = nc.dram_tensor("bqk", [HLOC, 2, 128, 1], F32, kind="ExternalInput")
    bv = nc.dram_tensor("bv", [1, CW], F32, kind="ExternalInput")
    # w_out rows permuted: wout[h][i] = w_out[512*i + 128*h : +128, cols]
    wout = nc.dram_tensor("wout", [HLOC, 4, 128, CW], BF, kind="ExternalInput")
    bout = nc.dram_tensor("bout", [1, CW], F32, kind="ExternalInput")
    out = nc.dram_tensor("out", [S, CW], F32, kind="ExternalOutput")

    # per (head, token-half) AG buffers; half 1 = tokens 1024:2048 (q-blocks
    # 3,2 -- computed first), half 0 = tokens 0:1024
    ag_in = {(h, hf): nc.dram_tensor(f"ag_in{h}_{hf}", [128, 1024], BF,
                                     kind="Internal")
             for h in range(HLOC) for hf in range(2)}
    ag_out = {(h, hf): nc.dram_tensor(f"ag_out{h}_{hf}", [512, 1024], BF,
                                      kind="Internal")
              for h in range(HLOC) for hf in range(2)}

    with tile.TileContext(nc) as tc:
        with (
            tc.tile_pool(name="const", bufs=1) as constp,
            tc.tile_pool(name="pers", bufs=1) as pers,
            tc.tile_pool(name="work", bufs=2) as work,
            tc.tile_pool(name="psum", bufs=2, space="PSUM") as psum,
        ):
            # ---- constants ----
            ones = constp.tile([128, 1], BF, name="ones")
            nc.gpsimd.memset(ones[:], 1.0)

            # Pair masks for the 4 diagonal k-subtiles, packed two subtiles
            # wide: pairmask[m][:, 512*sub + qq] keeps where
            # qq >= kk + 128*(2m+sub).
            pairmasks = []
            for pm in range(2):
                m = constp.tile([128, 1024], BF, name=f"pmask{pm}",
                                tag=f"pmask{pm}")
                nc.gpsimd.memset(m[:], 1.0)
                for sub in range(2):
                    nc.gpsimd.affine_select(
                        out=m[:, sub * 512:(sub + 1) * 512],
                        in_=m[:, sub * 512:(sub + 1) * 512],
                        compare_op=mybir.AluOpType.is_ge, fill=0.0,
                        base=-128 * (2 * pm + sub), channel_multiplier=-1,
                        pattern=[[1, 512]],
                    )
                pairmasks.append(m)

            bout_sb = constp.tile([1, CW], F32, name="bout_sb")
            nc.sync.dma_start(bout_sb[:], bout[:])
            bias_bc = constp.tile([128, CW], F32, name="bias_bc")
            nc.gpsimd.partition_broadcast(bias_bc[:], bout_sb[:], channels=128)

            bv_sb = constp.tile([1, CW], F32, name="bv_sb")
            nc.sync.dma_start(bv_sb[:], bv[:])
            vbias_bc = constp.tile([128, CW], F32, name="vbias_bc")
            nc.gpsimd.partition_broadcast(vbias_bc[:], bv_sb[:], channels=128)

            bqk_sb = {}
            for h in range(HLOC):
                for qk in range(2):
                    t = constp.tile([128, 1], F32, name=f"bqk{h}{qk}",
                                    tag=f"bqk{h}{qk}")
                    nc.sync.dma_start(t[:], bqk[h, qk])
                    bqk_sb[(h, qk)] = t

            # ---- persistent v tiles ----
            vt = [pers.tile([128, CW], BF, name=f"v{t}", tag=f"v{t}")
                  for t in range(16)]

            # ---- loads: wv panels first, then xT tb-major ----
            wv_sb = []
            for dt in range(DT):
                wvp = work.tile([128, CW], BF, name=f"wvp{dt}", tag="p512",
                                bufs=17)
                nc.sync.dma_start(wvp[:], wv[dt])
                wv_sb.append(wvp)
            xt_tiles = {}
            for tb in range(TB):
                for dt in range(DT):
                    t = work.tile([128, 512], BF, name=f"xt_{dt}_{tb}",
                                  tag="xT", bufs=68)
                    nc.sync.dma_start(
                        t[:], xT[dt * 128:(dt + 1) * 128, tb * 512:(tb + 1) * 512])
                    xt_tiles[(dt, tb)] = t

            # ---- v projection: v[t] = x @ wv  ([tok, vcol], xT stationary) ----
            for t in range(16):
                tb, j = t // 4, t % 4
                acc = psum.tile([128, CW], F32, name="acc_v", tag="acc", bufs=2)
                for dt in range(DT):
                    nc.tensor.matmul(
                        acc[:],
                        xt_tiles[(dt, tb)][:, j * 128:(j + 1) * 128],
                        wv_sb[dt][:],
                        start=(dt == 0), stop=(dt == DT - 1),
                    )
                nc.vector.tensor_tensor(vt[t][:], acc[:], vbias_bc[:],
                                        mybir.AluOpType.add)

            # ---- per-head q/k projection ([col, tok] transposed) ----
            def qk_proj(h):
                dests = {}
                for qk in range(2):
                    wts = []
                    for dt in range(DT):
                        wt = work.tile([128, 128], BF, name=f"w_{h}_{qk}_{dt}",
                                       tag="w", bufs=24)
                        nc.sync.dma_start(wt[:], wqk[h, qk, dt])
                        wts.append(wt)
                    dest = work.tile([128, S], BF, name=f"qkT_{h}_{qk}",
                                     tag="qkT", bufs=4)
                    for tb in range(TB):
                        acc = psum.tile([128, 512], F32, name="acc_qk",
                                        tag="acc", bufs=2)
                        for dt in range(DT):
                            nc.tensor.matmul(
                                acc[:], wts[dt][:], xt_tiles[(dt, tb)][:],
                                start=(dt == 0), stop=(dt == DT - 1),
                            )
                        nc.scalar.activation(
                            dest[:, tb * 512:(tb + 1) * 512], acc[:],
                            mybir.ActivationFunctionType.Identity,
                            bias=bqk_sb[(h, qk)][:], scale=1.0,
                        )
                    dests[qk] = dest
                return dests[0], dests[1]

            # ---- attention for one head (q-blocks descending) + its AG ----
            def attention_head(h, qTh, kTh):
                for qb in (3, 2, 1, 0):
                    nk = 4 * qb + 4
                    y_ps = psum.tile([128, 512], F32, name="y_ps", tag="y")
                    esum = work.tile([128, 1024], BF, name="esum", tag="esum",
                                     bufs=2)
                    prev = None

                    def flush(prev_pair):
                        e, pr = prev_pair
                        for s_ in range(2):
                            kt = 2 * pr + s_
                            nc.tensor.matmul(
                                y_ps[:],
                                vt[kt][:, h * 128:(h + 1) * 128],
                                e[:, s_ * 512:(s_ + 1) * 512],
                                start=(kt == 0), stop=(kt == nk - 1),
                            )
                        if pr == 0:
                            nc.vector.tensor_copy(esum[:], e[:])
                        else:
                            nc.vector.tensor_tensor(esum[:], esum[:], e[:],
                                                    mybir.AluOpType.add)

                    for pr in range(nk // 2):
                        sc = psum.tile([128, 1024], F32, name="sc", tag="s",
                                       bufs=2)
                        for s_ in range(2):
                            kt = 2 * pr + s_
                            nc.tensor.matmul(
                                sc[:, s_ * 512:(s_ + 1) * 512],
                                kTh[:, kt * 128:(kt + 1) * 128],
                                qTh[:, qb * 512:(qb + 1) * 512],
                                start=True, stop=True,
                            )
                        e = work.tile([128, 1024], BF, name="expT", tag="expT",
                                      bufs=4)
                        nc.scalar.activation(
                            e[:], sc[:], mybir.ActivationFunctionType.Exp,
                            scale=SCALE,
                        )
                        pm = pr - (nk // 2 - 2)
                        if pm >= 0:
                            nc.vector.tensor_tensor(e[:], e[:],
                                                    pairmasks[pm][:],
                                                    mybir.AluOpType.mult)
                        if prev is not None:
                            flush(prev)
                        prev = (e, pr)
                    flush(prev)

                    esum_f = work.tile([128, 512], BF, name="esum_f",
                                       tag="esum_f", bufs=2)
                    nc.vector.tensor_tensor(esum_f[:], esum[:, 0:512],
                                            esum[:, 512:1024],
                                            mybir.AluOpType.add)
                    sum_ps = psum.tile([1, 512], F32, name="sum_ps", tag="y")
                    nc.tensor.matmul(sum_ps[:], ones[:], esum_f[:],
                                     start=True, stop=True)
                    recip = work.tile([1, 512], F32, name="recip", tag="recip",
                                      bufs=2)
                    nc.vector.reciprocal_approx_fast(recip[:], sum_ps[:])
                    rbc = work.tile([128, 512], F32, name="rbc", tag="rbc",
                                    bufs=2)
                    nc.gpsimd.partition_broadcast(rbc[:], recip[:], channels=128)
                    ynorm = work.tile([128, 512], BF, name="ynorm", tag="ynorm",
                                      bufs=3)
                    nc.vector.tensor_tensor(ynorm[:], y_ps[:], rbc[:],
                                            mybir.AluOpType.mult)
                    hf, co = qb // 2, (qb % 2) * 512
                    nc.sync.dma_start(
                        ag_in[(h, hf)][:, co:co + 512], ynorm[:])
                    if qb in (2, 0):
                        nc.gpsimd.collective_compute(
                            "AllGather", mybir.AluOpType.bypass,
                            replica_groups=GROUPS,
                            ins=[ag_in[(h, hf)].ap()],
                            outs=[ag_out[(h, hf)].ap()],
                        )

            # ---- out-projection partial pass for head-chunk h ----
            wout_sb = {}

            def load_wout():
                for h in range(HLOC):
                    for i in range(4):
                        t = work.tile([128, CW], BF, name=f"wout{h}{i}",
                                      tag="p512", bufs=17)
                        nc.sync.dma_start(t[:], wout[h, i])
                        wout_sb[(h, i)] = t

            part = {}

            def outproj_pass(h):
                for tc_ in (2, 3, 0, 1):
                    hf, co = tc_ // 2, (tc_ % 2) * 512
                    ygt = []
                    for i in range(4):
                        t = work.tile([128, 512], BF, name=f"yg_{h}_{tc_}_{i}",
                                      tag="ygt", bufs=10)
                        nc.sync.dma_start(
                            t[:], ag_out[(h, hf)][i * 128:(i + 1) * 128,
                                                  co:co + 512])
                        ygt.append(t)
                    for j in range(4):
                        t = tc_ * 4 + j
                        acc = psum.tile([128, CW], F32, name="acc_o",
                                        tag="acc", bufs=2)
                        for i in range(4):
                            nc.tensor.matmul(
                                acc[:],
                                ygt[i][:, j * 128:(j + 1) * 128],
                                wout_sb[(h, i)][:],
                                start=(i == 0), stop=(i == 3),
                            )
                        if h == 0:
                            p = work.tile([128, CW], BF, name=f"part{t}",
                                          tag=f"part{t}", bufs=1)
                            part[t] = p
                            nc.vector.tensor_tensor(p[:], acc[:], bias_bc[:],
                                                    mybir.AluOpType.add)
                        elif h < HLOC - 1:
                            nc.vector.tensor_tensor(part[t][:], part[t][:],
                                                    acc[:],
                                                    mybir.AluOpType.add)
                        else:
                            osb = work.tile([128, CW], F32, name="osb",
                                            tag="osb", bufs=3)
                            nc.vector.tensor_tensor(osb[:], part[t][:], acc[:],
                                                    mybir.AluOpType.add)
                            nc.sync.dma_start(
                                out[t * 128:(t + 1) * 128, :], osb[:])

            # ---- head pipeline: attention first (AG trigger asap), then
            # next head's projection, then the pass for the landed AG ----
            qk_tiles = qk_proj(0)
            load_wout()
            for h in range(HLOC):
                attention_head(h, *qk_tiles)
                qk_tiles = qk_proj(h + 1) if h + 1 < HLOC else None
                if h > 1:
                    outproj_pass(h - 2)
            outproj_pass(HLOC - 2)
            outproj_pass(HLOC - 1)

    nc.compile()
    return nc


def _prep_inputs(x, w_qkv, b_qkv, w_out, b_out):
    """Host-side sharding/layout. Returns in_maps for the 8 cores."""
    bf16 = ml_dtypes.bfloat16
    x = np.asarray(x, dtype=np.float32)
    w_qkv = np.asarray(w_qkv, dtype=np.float32)
    b_qkv = np.asarray(b_qkv, dtype=np.float32)
    w_out = np.asarray(w_out, dtype=np.float32)
    b_out = np.asarray(b_out, dtype=np.float32)

    xT_b = [np.ascontiguousarray(x[b].T).astype(bf16) for b in range(B)]

    in_maps = []
    for c in range(8):
        b, g = c // 4, c % 4
        cols = slice(CW * g, CW * (g + 1))

        # wqk[h][0]=q, [1]=k tiles for global head 4g+h, [dt, 128, 128]
        wqk = np.empty((HLOC, 2, DT, 128, 128), np.float32)
        bqk = np.empty((HLOC, 2, 128, 1), np.float32)
        for h in range(HLOC):
            gh = 4 * g + h
            for qk in range(2):
                wcol = w_qkv[:, qk * D + 128 * gh: qk * D + 128 * (gh + 1)]
                wqk[h, qk] = wcol.reshape(DT, 128, 128)
                bqk[h, qk, :, 0] = b_qkv[qk * D + 128 * gh: qk * D + 128 * (gh + 1)]

        wv_ = w_qkv[:, 2 * D:3 * D][:, cols]
        bv_ = b_qkv[2 * D:3 * D][cols]

        # w_out rows permuted to the AG's rank-major order per head chunk
        wout_loc = w_out[:, cols]
        wout_t = np.empty((HLOC, 4, 128, CW), np.float32)
        for h in range(HLOC):
            for i in range(4):
                wout_t[h, i] = wout_loc[512 * i + 128 * h: 512 * i + 128 * (h + 1), :]

        in_maps.append({
            "xT": xT_b[b],
            "wqk": np.ascontiguousarray(wqk).astype(bf16),
            "wv": np.ascontiguousarray(wv_.reshape(DT, 128, CW)).astype(bf16),
            "bqk": np.ascontiguousarray(bqk),
            "bv": np.ascontiguousarray(bv_.reshape(1, CW)),
            "wout": np.ascontiguousarray(wout_t).astype(bf16),
            "bout": np.ascontiguousarray(b_out[cols].reshape(1, CW)),
        })
    return in_maps


def kernel(x, w_qkv, b_qkv, w_out, b_out, _trace=False, _trace_kwargs=None):
    from concourse.bass_utils import run_bass_kernel_spmd

    if "nc" not in _cache:
        _cache["nc"] = _build()
    nc = _cache["nc"]

    in_maps = _prep_inputs(x, w_qkv, b_qkv, w_out, b_out)
    res = run_bass_kernel_spmd(
        nc, in_maps, core_ids=list(range(8)),
        trace=_trace, **(_trace_kwargs or {}),
    )

    out = np.empty((B, S, D), dtype=np.float32)
    for c in range(8):
        b, g = c // 4, c % 4
        out[b][:, CW * g:CW * (g + 1)] = res.results[c]["out"]
    kernel.last_result = res
    return out
